# revision 7
# baseline (speedup 1.0000x reference)
# Trainium2 Bass kernel for the DVAE encoder (nn_DVAE_24850680775463).
#
# Sharding: pure data-parallel. B=1024 graphs -> 8 cores x 128 graphs.
# Per core, the 128 graphs sit on the 128 SBUF partitions and the whole
# 16-vertex sequential scan runs on-chip.
#
# Key restructurings vs the reference:
#  * gate/mapper products are computed once per vertex (incremental cache)
#    instead of for all 16 candidate predecessors every step; the vertex-id
#    one-hot contribution folds into a per-vertex bias row.
#  * the adjacency-weighted message H_v = sum_n adj[b,v,n] * gm[b,n,:] is
#    computed on the TensorEngine as PSUM-accumulated matmuls with
#    diag(adj[:,v,n]) as the stationary operand.
#  * GRU x-side contributions and all biases enter via small extra matmuls
#    (one-hot / ones rows) accumulating into the same PSUM banks as the
#    h-side matmuls, so gate pre-activations come out of PSUM fully summed.
#  * (1-z) is computed as sigmoid(-Z) to shorten the combine chain.
# Matmuls run as float32r (fp32 data, full-rate PE).

import os
import numpy as np

import concourse.bass as bass
import concourse.tile as tile
from concourse import bacc, mybir
from concourse.bass_utils import run_bass_kernel_spmd

AF = mybir.ActivationFunctionType
F32 = mybir.dt.float32

NCORES = 8
B, NV, NVT, FS, HS, NZ = 1024, 16, 16, 32, 512, 64
P = B // NCORES            # 128 graphs per core
G3 = 3 * HS                # 1536
K1 = NVT + 1               # 17  (one-hot + ones row)
K2 = FS + 1                # 33  (params + ones row)
KC = HS // 128             # 4 contraction chunks of the hidden dim

MMDT = {"f32r": mybir.dt.float32r, "f32": mybir.dt.float32,
        "bf16": mybir.dt.bfloat16}[os.environ.get("DVAE_MMDT", "f32r")]


def _mm(ap):
    return ap


def build_bass():
    nc = bacc.Bacc("TRN2", target_bir_lowering=False, debug=False)

    def inp(name, shape, dt=None):
        return nc.dram_tensor(name, shape, dt or MMDT,
                              kind="ExternalInput").ap()

    d = {
        "wht_t": inp("wht_t", [128, KC * G3]),
        "wht_p": inp("wht_p", [128, KC * G3]),
        "w1x":   inp("w1x",   [K1, G3]),
        "w2x":   inp("w2x",   [K2, G3]),
        "xt1":   inp("xt1",   [K1, NV * P]),
        "xp1":   inp("xp1",   [K2, NV * P]),
        "bhn2":  inp("bhn2",  [1, 2 * HS]),
        "wgm":   inp("wgm",   [128, KC * 2 * HS]),
        "bgm":   inp("bgm",   [NV, 2 * HS]),
        "vsel":  inp("vsel",  [NV, NV * P]),
        "adjt":  inp("adjt",  [P, NV * NV], F32),
        "wfc":   inp("wfc",   [128, KC * 2 * NZ]),
        "bfc":   inp("bfc",   [1, 2 * NZ]),
        "eye":   inp("eye",   [128, 128], F32),
        "ones1": inp("ones1", [1, 128]),
    }
    out_ap = nc.dram_tensor("out", [P, 2 * NZ], mybir.dt.float32, kind="ExternalOutput").ap()

    with tile.TileContext(nc) as tc:
        _body(tc, d, out_ap)
    nc.compile()
    return nc


def _body(tc, d, out_ap):
    nc = tc.nc
    from contextlib import ExitStack
    with ExitStack() as ctx:
        wp = ctx.enter_context(tc.tile_pool(name="w", bufs=1))
        sp = ctx.enter_context(tc.tile_pool(name="s", bufs=1))
        dgp = ctx.enter_context(tc.tile_pool(name="dg", bufs=4))
        gmc = ctx.enter_context(tc.tile_pool(name="gmc", bufs=1))
        ps_h = ctx.enter_context(tc.tile_pool(name="psh", bufs=2, space="PSUM"))
        ps_g = ctx.enter_context(tc.tile_pool(name="psg", bufs=4, space="PSUM"))
        ps_m = ctx.enter_context(tc.tile_pool(name="psm", bufs=1, space="PSUM"))

        # ---- persistent weights / constants ----
        W = {}
        for name, ap in d.items():
            t = wp.tile(list(ap.shape), ap.dtype, tag=name)
            nc.sync.dma_start(t[:], ap[:, :])
            W[name] = t

        wht = {0: W["wht_t"], 1: W["wht_p"]}
        wx = {0: W["w1x"], 1: W["w2x"]}
        xs = {0: W["xt1"], 1: W["xp1"]}
        kx = {0: K1, 1: K2}
        eye = W["eye"]
        adjt = W["adjt"]

        gm_sb = []          # cached gate*mapped per vertex, [P, HS] each

        def transpose512(src_sb, tag):
            """[128,512] batch-major -> feature-major (4 chunks side by side)."""
            tp = ps_m.tile([128, HS], F32, tag="psm")
            for c in range(KC):
                nc.tensor.transpose(tp[:, c * 128:(c + 1) * 128],
                                    src_sb[:, c * 128:(c + 1) * 128], eye[:])
            dst = sp.tile([128, HS], MMDT, tag=tag)
            nc.scalar.copy(dst[:], tp[:])
            return dst

        def gru_mms(g, v, HT):
            """Emit all matmuls of GRU g (0=type,1=param) for vertex v.
            HT: feature-major hidden state [128, 512] or None (h=0)."""
            K = kx[g]
            xl = _mm(xs[g][:K, v * P:(v + 1) * P])      # lhsT [K,128]
            wxr = wx[g]
            w = wht[g]
            R = ps_g.tile([128, HS], F32, tag="g")
            Z = ps_g.tile([128, HS], F32, tag="g")
            NI = ps_g.tile([128, HS], F32, tag="g")
            NH = ps_g.tile([128, HS], F32, tag="g")
            # emission in consumption order: R, NH, NI, Z
            nc.tensor.matmul(R[:], xl, _mm(wxr[:K, 0:HS]),
                             start=True, stop=HT is None)
            nc.tensor.matmul(NH[:], _mm(W["ones1"][:]), _mm(W["bhn2"][0:1, g * HS:(g + 1) * HS]),
                             start=True, stop=HT is None)
            nc.tensor.matmul(NI[:], xl, _mm(wxr[:K, 2 * HS:G3]),
                             start=True, stop=True)
            nc.tensor.matmul(Z[:], xl, _mm(wxr[:K, HS:2 * HS]),
                             start=True, stop=HT is None)
            if HT is not None:
                for c in range(KC):
                    hl = _mm(HT[:, c * 128:(c + 1) * 128])
                    last = c == KC - 1
                    nc.tensor.matmul(R[:], hl, _mm(w[:, c * G3:c * G3 + HS]),
                                     start=False, stop=last)
                    nc.tensor.matmul(NH[:], hl, _mm(w[:, c * G3 + 2 * HS:(c + 1) * G3]),
                                     start=False, stop=last)
                    nc.tensor.matmul(Z[:], hl, _mm(w[:, c * G3 + HS:c * G3 + 2 * HS]),
                                     start=False, stop=last)
            return R, Z, NI, NH

        def gru_ew(R, Z, NI, NH, h_sb):
            """Elementwise GRU combine. h_sb: batch-major h or None. -> hv"""
            r = sp.tile([128, HS], F32, tag="r")
            nc.scalar.activation(r[:], R[:], AF.Sigmoid)
            z = zh = None
            if h_sb is not None:
                z = sp.tile([128, HS], F32, tag="z")
                nc.scalar.activation(z[:], Z[:], AF.Sigmoid)
            zc = sp.tile([128, HS], F32, tag="zc")
            nc.scalar.activation(zc[:], Z[:], AF.Sigmoid, scale=-1.0)
            if h_sb is not None:
                zh = sp.tile([128, HS], F32, tag="zh")
                nc.vector.tensor_mul(zh[:], z[:], h_sb[:])
            rhn = sp.tile([128, HS], F32, tag="rhn")
            nc.vector.tensor_mul(rhn[:], r[:], NH[:])
            npre = sp.tile([128, HS], F32, tag="npre")
            nc.vector.tensor_add(npre[:], rhn[:], NI[:])
            n = sp.tile([128, HS], F32, tag="n")
            nc.scalar.activation(n[:], npre[:], AF.Tanh)
            zn = sp.tile([128, HS], F32, tag="zn")
            nc.vector.tensor_mul(zn[:], zc[:], n[:])
            if h_sb is None:
                return zn
            hv = sp.tile([128, HS], F32, tag="hv")
            nc.vector.tensor_add(hv[:], zn[:], zh[:])
            return hv

        for v in range(NV):
            if v == 0:
                hH = None
                HT = None
            else:
                # message H_v = sum_{n<v} diag(adj[:,v,n]) @ gm_n  (PSUM accum)
                H = ps_h.tile([128, HS], F32, tag="H")
                for n in range(v):
                    dg = dgp.tile([128, 128], MMDT, tag="diag")
                    col = v * NV + n
                    nc.vector.tensor_scalar_mul(dg[:], eye[:],
                                                adjt[:, col:col + 1])
                    nc.tensor.matmul(H[:], _mm(dg[:]), _mm(gm_sb[n][:]),
                                     start=(n == 0), stop=(n == v - 1))
                hH = sp.tile([128, HS], F32, tag="hH")
                nc.scalar.copy(hH[:], H[:])
                HT = transpose512(hH, "HT")

            R, Z, NI, NH = gru_mms(0, v, HT)
            hv1 = gru_ew(R, Z, NI, NH, hH)
            hv1T = transpose512(hv1, "hv1T")
            R, Z, NI, NH = gru_mms(1, v, hv1T)
            hv = gru_ew(R, Z, NI, NH, hv1)
            hvT = transpose512(hv, "hvT")

            if v < NV - 1:
                # gate/mapper for vertex v (feeds future messages)
                gmp = ps_m.tile([128, 2 * HS], F32, tag="psm")
                vl = _mm(W["vsel"][:, v * P:(v + 1) * P])
                nc.tensor.matmul(gmp[:, 0:HS], vl, _mm(W["bgm"][:, 0:HS]),
                                 start=True, stop=False)
                nc.tensor.matmul(gmp[:, HS:2 * HS], vl, _mm(W["bgm"][:, HS:2 * HS]),
                                 start=True, stop=False)
                for c in range(KC):
                    hl = _mm(hvT[:, c * 128:(c + 1) * 128])
                    last = c == KC - 1
                    nc.tensor.matmul(gmp[:, 0:HS], hl,
                                     _mm(W["wgm"][:, c * 2 * HS:c * 2 * HS + HS]),
                                     start=False, stop=last)
                    nc.tensor.matmul(gmp[:, HS:2 * HS], hl,
                                     _mm(W["wgm"][:, c * 2 * HS + HS:(c + 1) * 2 * HS]),
                                     start=False, stop=last)
                gate = sp.tile([128, HS], F32, tag="gate")
                nc.scalar.activation(gate[:], gmp[:, 0:HS], AF.Sigmoid)
                gmt = gmc.tile([128, HS], MMDT, tag=f"gm{v}")
                nc.vector.tensor_mul(gmt[:], gate[:], gmp[:, HS:2 * HS])
                gm_sb.append(gmt)
            else:
                # final FC: out = Hg @ Wfc + bfc   (mu | logvar)
                fcp = ps_m.tile([128, 2 * NZ], F32, tag="psm")
                nc.tensor.matmul(fcp[:], _mm(W["ones1"][:]), _mm(W["bfc"][:, :]),
                                 start=True, stop=False)
                for c in range(KC):
                    nc.tensor.matmul(fcp[:], _mm(hvT[:, c * 128:(c + 1) * 128]),
                                     _mm(W["wfc"][:, c * 2 * NZ:(c + 1) * 2 * NZ]),
                                     start=False, stop=(c == KC - 1))
                fc = sp.tile([128, 2 * NZ], F32, tag="fc")
                nc.scalar.copy(fc[:], fcp[:])
                nc.sync.dma_start(out_ap[:, :], fc[:])


def _host_prep(types, params, adj, gt_wi, gt_wh, gt_bi, gt_bh,
               gp_wi, gp_wh, gp_bi, gp_bh, gate_w, gate_b, mapper_w,
               fc1_w, fc1_b, fc2_w, fc2_b):
    """Pure layout prep: transposes/reshapes/one-hot + per-core sharding."""
    f = np.float32

    def chunked(a):  # [512, X] -> [128, 4*X] with K-chunks side by side
        X = a.shape[1]
        return np.ascontiguousarray(
            a.reshape(KC, 128, X).transpose(1, 0, 2).reshape(128, KC * X)).astype(f)

    b1 = np.concatenate([(gt_bi + gt_bh)[:2 * HS], gt_bi[2 * HS:]])
    b2 = np.concatenate([(gp_bi + gp_bh)[:2 * HS], gp_bi[2 * HS:]])
    shared = {
        "wht_t": chunked(gt_wh.T.astype(f)),
        "wht_p": chunked(gp_wh.T.astype(f)),
        "w1x": np.concatenate([gt_wi.T, b1[None, :]], 0).astype(f),
        "w2x": np.concatenate([gp_wi.T, b2[None, :]], 0).astype(f),
        "bhn2": np.concatenate([gt_bh[2 * HS:], gp_bh[2 * HS:]])[None, :].astype(f),
        "wgm": chunked(np.concatenate([gate_w[:, :HS].T, mapper_w[:, :HS].T], 1)),
        "bgm": np.stack([np.concatenate([gate_b + gate_w[:, HS + v],
                                         mapper_w[:, HS + v]])
                         for v in range(NV)]).astype(f),
        "vsel": np.repeat(np.eye(NV, dtype=f), P, axis=1),
        "wfc": chunked(np.concatenate([fc1_w.T, fc2_w.T], 1).astype(f)),
        "bfc": np.concatenate([fc1_b, fc2_b])[None, :].astype(f),
        "eye": np.eye(128, dtype=f),
        "ones1": np.ones((1, 128), f),
    }
    oh = (types[:, :, None] == np.arange(NVT)[None, None, :]).astype(f)  # [B,NV,NVT]
    in_maps = []
    for c in range(NCORES):
        s = slice(c * P, (c + 1) * P)
        xt = oh[s].transpose(2, 1, 0).reshape(NVT, NV * P)           # [16, NV*P]
        xt1 = np.concatenate([xt, np.ones((1, NV * P), f)], 0)
        xp = params[s].transpose(2, 1, 0).reshape(FS, NV * P).astype(f)
        xp1 = np.concatenate([xp, np.ones((1, NV * P), f)], 0)
        m = dict(shared)
        m["xt1"] = np.ascontiguousarray(xt1)
        m["xp1"] = np.ascontiguousarray(xp1)
        m["adjt"] = np.ascontiguousarray(adj[s].reshape(P, NV * NV)).astype(f)
        in_maps.append(m)
    return in_maps


_NC_CACHE = {}


def _get_nc():
    key = str(MMDT)
    if key not in _NC_CACHE:
        _NC_CACHE[key] = build_bass()
    return _NC_CACHE[key]


def kernel(**inputs):
    np_inputs = {k: np.asarray(v) for k, v in inputs.items()}
    in_maps = _host_prep(**np_inputs)
    nc = _get_nc()
    res = run_bass_kernel_spmd(nc, in_maps, core_ids=list(range(NCORES)),
                               **_RUN_KWARGS)
    out = np.concatenate([res.results[c]["out"] for c in range(NCORES)], 0)
    _LAST_RESULT.clear()
    _LAST_RESULT.append(res)
    return out[:, :NZ], out[:, NZ:]


# test.py can set these to enable tracing / inspect results
_RUN_KWARGS = {}
_LAST_RESULT = []


# revision 8
# speedup vs baseline: 1.0812x; 1.0812x over previous
# Trainium2 Bass kernel for the DVAE encoder (nn_DVAE_24850680775463).
#
# Sharding: pure data-parallel. B=1024 graphs -> 8 cores x 128 graphs.
# Per core, the 128 graphs sit on the 128 SBUF partitions and the whole
# 16-vertex sequential scan runs on-chip.
#
# Key restructurings vs the reference:
#  * gate/mapper products are computed once per vertex (incremental cache)
#    instead of for all 16 candidate predecessors every step; the vertex-id
#    one-hot contribution folds into a per-vertex bias row.
#  * the adjacency-weighted message H_v = sum_n adj[b,v,n] * gm[b,n,:] is
#    computed on the TensorEngine as PSUM-accumulated matmuls with
#    diag(adj[:,v,n]) as the stationary operand.
#  * GRU x-side contributions and all biases enter via small extra matmuls
#    (one-hot / ones rows) accumulating into the same PSUM banks as the
#    h-side matmuls, so gate pre-activations come out of PSUM fully summed.
#  * (1-z) is computed as sigmoid(-Z) to shorten the combine chain.
# Matmuls run as float32r (fp32 data, full-rate PE).

import os
import numpy as np

import concourse.bass as bass
import concourse.tile as tile
from concourse import bacc, mybir
from concourse.bass_utils import run_bass_kernel_spmd

AF = mybir.ActivationFunctionType
F32 = mybir.dt.float32

NCORES = 8
B, NV, NVT, FS, HS, NZ = 1024, 16, 16, 32, 512, 64
P = B // NCORES            # 128 graphs per core
G3 = 3 * HS                # 1536
K1 = NVT + 1               # 17  (one-hot + ones row)
K2 = FS + 1                # 33  (params + ones row)
KC = HS // 128             # 4 contraction chunks of the hidden dim

MMDT = {"f32r": mybir.dt.float32r, "f32": mybir.dt.float32,
        "bf16": mybir.dt.bfloat16}[os.environ.get("DVAE_MMDT", "f32r")]


def _mm(ap):
    return ap


def build_bass():
    nc = bacc.Bacc("TRN2", target_bir_lowering=False, debug=False)

    def inp(name, shape, dt=None):
        return nc.dram_tensor(name, shape, dt or MMDT,
                              kind="ExternalInput").ap()

    d = {
        "wht_t": inp("wht_t", [128, KC * G3]),
        "wht_p": inp("wht_p", [128, KC * G3]),
        "w1x":   inp("w1x",   [K1, G3]),
        "w2x":   inp("w2x",   [K2, G3]),
        "xt1":   inp("xt1",   [K1, NV * P]),
        "xp1":   inp("xp1",   [K2, NV * P]),
        "bhn2":  inp("bhn2",  [1, 2 * HS]),
        "wgm":   inp("wgm",   [128, KC * 2 * HS]),
        "bgm":   inp("bgm",   [NV, 2 * HS]),
        "vsel":  inp("vsel",  [NV, NV * P]),
        "adjt":  inp("adjt",  [P, NV * NV], F32),
        "wfc":   inp("wfc",   [128, KC * 2 * NZ]),
        "bfc":   inp("bfc",   [1, 2 * NZ]),
        "eye":   inp("eye",   [128, 128], F32),
        "ones1": inp("ones1", [1, 128]),
    }
    out_ap = nc.dram_tensor("out", [P, 2 * NZ], mybir.dt.float32, kind="ExternalOutput").ap()

    with tile.TileContext(nc) as tc:
        _body(tc, d, out_ap)
    nc.compile()
    return nc


def _body(tc, d, out_ap):
    nc = tc.nc
    from contextlib import ExitStack
    with ExitStack() as ctx:
        wp = ctx.enter_context(tc.tile_pool(name="w", bufs=1))
        sp = ctx.enter_context(tc.tile_pool(name="s", bufs=1))
        dgp = ctx.enter_context(tc.tile_pool(name="dg", bufs=4))
        gmc = ctx.enter_context(tc.tile_pool(name="gmc", bufs=1))
        ps_h = ctx.enter_context(tc.tile_pool(name="psh", bufs=2, space="PSUM"))
        ps_g = ctx.enter_context(tc.tile_pool(name="psg", bufs=4, space="PSUM"))
        ps_m = ctx.enter_context(tc.tile_pool(name="psm", bufs=1, space="PSUM"))

        # ---- persistent weights / constants ----
        W = {}
        for name, ap in d.items():
            t = wp.tile(list(ap.shape), ap.dtype, tag=name)
            nc.sync.dma_start(t[:], ap[:, :])
            W[name] = t

        wht = {0: W["wht_t"], 1: W["wht_p"]}
        wx = {0: W["w1x"], 1: W["w2x"]}
        xs = {0: W["xt1"], 1: W["xp1"]}
        kx = {0: K1, 1: K2}
        eye = W["eye"]
        adjt = W["adjt"]

        gm_sb = []          # cached gate*mapped per vertex, [P, HS] each

        def transpose512(src_sb, tag):
            """[128,512] batch-major -> feature-major (4 chunks side by side)."""
            tp = ps_m.tile([128, HS], F32, tag="psm")
            for c in range(KC):
                nc.tensor.transpose(tp[:, c * 128:(c + 1) * 128],
                                    src_sb[:, c * 128:(c + 1) * 128], eye[:])
            dst = sp.tile([128, HS], MMDT, tag=tag)
            nc.scalar.copy(dst[:], tp[:])
            return dst

        def gru_mms(g, v, HT):
            """Emit all matmuls of GRU g (0=type,1=param) for vertex v.
            HT: feature-major hidden state [128, 512] or None (h=0)."""
            K = kx[g]
            xl = _mm(xs[g][:K, v * P:(v + 1) * P])      # lhsT [K,128]
            wxr = wx[g]
            w = wht[g]
            R = ps_g.tile([128, HS], F32, tag="g")
            Z = ps_g.tile([128, HS], F32, tag="g")
            NI = ps_g.tile([128, HS], F32, tag="g")
            NH = ps_g.tile([128, HS], F32, tag="g")
            # emission in consumption order: R, NH, NI, Z
            nc.tensor.matmul(R[:], xl, _mm(wxr[:K, 0:HS]),
                             start=True, stop=HT is None)
            nc.tensor.matmul(NH[:], _mm(W["ones1"][:]), _mm(W["bhn2"][0:1, g * HS:(g + 1) * HS]),
                             start=True, stop=HT is None)
            nc.tensor.matmul(NI[:], xl, _mm(wxr[:K, 2 * HS:G3]),
                             start=True, stop=True)
            nc.tensor.matmul(Z[:], xl, _mm(wxr[:K, HS:2 * HS]),
                             start=True, stop=HT is None)
            if HT is not None:
                for c in range(KC):
                    hl = _mm(HT[:, c * 128:(c + 1) * 128])
                    last = c == KC - 1
                    nc.tensor.matmul(R[:], hl, _mm(w[:, c * G3:c * G3 + HS]),
                                     start=False, stop=last)
                    nc.tensor.matmul(NH[:], hl, _mm(w[:, c * G3 + 2 * HS:(c + 1) * G3]),
                                     start=False, stop=last)
                    nc.tensor.matmul(Z[:], hl, _mm(w[:, c * G3 + HS:c * G3 + 2 * HS]),
                                     start=False, stop=last)
            return R, Z, NI, NH

        def gru_ew(R, Z, NI, NH, h_sb):
            """Elementwise GRU combine. h_sb: batch-major h or None. -> hv"""
            r = sp.tile([128, HS], F32, tag="r")
            nc.scalar.activation(r[:], R[:], AF.Sigmoid)
            z = zh = None
            if h_sb is not None:
                z = sp.tile([128, HS], F32, tag="z")
                nc.scalar.activation(z[:], Z[:], AF.Sigmoid)
            zc = sp.tile([128, HS], F32, tag="zc")
            nc.scalar.activation(zc[:], Z[:], AF.Sigmoid, scale=-1.0)
            if h_sb is not None:
                zh = sp.tile([128, HS], F32, tag="zh")
                nc.vector.tensor_mul(zh[:], z[:], h_sb[:])
            rhn = sp.tile([128, HS], F32, tag="rhn")
            nc.vector.tensor_mul(rhn[:], r[:], NH[:])
            npre = sp.tile([128, HS], F32, tag="npre")
            nc.vector.tensor_add(npre[:], rhn[:], NI[:])
            n = sp.tile([128, HS], F32, tag="n")
            nc.scalar.activation(n[:], npre[:], AF.Tanh)
            zn = sp.tile([128, HS], F32, tag="zn")
            nc.vector.tensor_mul(zn[:], zc[:], n[:])
            if h_sb is None:
                return zn
            hv = sp.tile([128, HS], F32, tag="hv")
            nc.vector.tensor_add(hv[:], zn[:], zh[:])
            return hv

        for v in range(NV):
            if v == 0:
                hH = None
                HT = None
            else:
                # message H_v = sum_{n<v} diag(adj[:,v,n]) @ gm_n  (PSUM accum)
                H = ps_h.tile([128, HS], F32, tag="H")
                for n in range(v):
                    dg = dgp.tile([128, 128], MMDT, tag="diag")
                    col = v * NV + n
                    nc.vector.tensor_scalar_mul(dg[:], eye[:],
                                                adjt[:, col:col + 1])
                    nc.tensor.matmul(H[:], _mm(dg[:]), _mm(gm_sb[n][:]),
                                     start=(n == 0), stop=(n == v - 1))
                hH = sp.tile([128, HS], F32, tag="hH")
                nc.scalar.copy(hH[:], H[:])
                HT = transpose512(hH, "HT")

            R, Z, NI, NH = gru_mms(0, v, HT)
            hv1 = gru_ew(R, Z, NI, NH, hH)
            hv1T = transpose512(hv1, "hv1T")
            R, Z, NI, NH = gru_mms(1, v, hv1T)
            hv = gru_ew(R, Z, NI, NH, hv1)
            hvT = transpose512(hv, "hvT")

            if v < NV - 1:
                # gate/mapper for vertex v (feeds future messages)
                gmp = ps_m.tile([128, 2 * HS], F32, tag="psm")
                vl = _mm(W["vsel"][:, v * P:(v + 1) * P])
                nc.tensor.matmul(gmp[:, 0:HS], vl, _mm(W["bgm"][:, 0:HS]),
                                 start=True, stop=False)
                nc.tensor.matmul(gmp[:, HS:2 * HS], vl, _mm(W["bgm"][:, HS:2 * HS]),
                                 start=True, stop=False)
                for c in range(KC):
                    hl = _mm(hvT[:, c * 128:(c + 1) * 128])
                    last = c == KC - 1
                    nc.tensor.matmul(gmp[:, 0:HS], hl,
                                     _mm(W["wgm"][:, c * 2 * HS:c * 2 * HS + HS]),
                                     start=False, stop=last)
                    nc.tensor.matmul(gmp[:, HS:2 * HS], hl,
                                     _mm(W["wgm"][:, c * 2 * HS + HS:(c + 1) * 2 * HS]),
                                     start=False, stop=last)
                gate = sp.tile([128, HS], F32, tag="gate")
                nc.scalar.activation(gate[:], gmp[:, 0:HS], AF.Sigmoid)
                gmt = gmc.tile([128, HS], MMDT, tag=f"gm{v}")
                nc.vector.tensor_mul(gmt[:], gate[:], gmp[:, HS:2 * HS])
                gm_sb.append(gmt)
            else:
                # final FC: out = Hg @ Wfc + bfc   (mu | logvar)
                fcp = ps_m.tile([128, 2 * NZ], F32, tag="psm")
                nc.tensor.matmul(fcp[:], _mm(W["ones1"][:]), _mm(W["bfc"][:, :]),
                                 start=True, stop=False)
                for c in range(KC):
                    nc.tensor.matmul(fcp[:], _mm(hvT[:, c * 128:(c + 1) * 128]),
                                     _mm(W["wfc"][:, c * 2 * NZ:(c + 1) * 2 * NZ]),
                                     start=False, stop=(c == KC - 1))
                fc = sp.tile([128, 2 * NZ], F32, tag="fc")
                nc.scalar.copy(fc[:], fcp[:])
                nc.sync.dma_start(out_ap[:, :], fc[:])


def _host_prep(types, params, adj, gt_wi, gt_wh, gt_bi, gt_bh,
               gp_wi, gp_wh, gp_bi, gp_bh, gate_w, gate_b, mapper_w,
               fc1_w, fc1_b, fc2_w, fc2_b):
    """Pure layout prep: transposes/reshapes/one-hot + per-core sharding."""
    f = np.float32

    def chunked(a):  # [512, X] -> [128, 4*X] with K-chunks side by side
        X = a.shape[1]
        return np.ascontiguousarray(
            a.reshape(KC, 128, X).transpose(1, 0, 2).reshape(128, KC * X)).astype(f)

    b1 = np.concatenate([(gt_bi + gt_bh)[:2 * HS], gt_bi[2 * HS:]])
    b2 = np.concatenate([(gp_bi + gp_bh)[:2 * HS], gp_bi[2 * HS:]])
    shared = {
        "wht_t": chunked(gt_wh.T.astype(f)),
        "wht_p": chunked(gp_wh.T.astype(f)),
        "w1x": np.concatenate([gt_wi.T, b1[None, :]], 0).astype(f),
        "w2x": np.concatenate([gp_wi.T, b2[None, :]], 0).astype(f),
        "bhn2": np.concatenate([gt_bh[2 * HS:], gp_bh[2 * HS:]])[None, :].astype(f),
        "wgm": chunked(np.concatenate([gate_w[:, :HS].T, mapper_w[:, :HS].T], 1)),
        "bgm": np.stack([np.concatenate([gate_b + gate_w[:, HS + v],
                                         mapper_w[:, HS + v]])
                         for v in range(NV)]).astype(f),
        "vsel": np.repeat(np.eye(NV, dtype=f), P, axis=1),
        "wfc": chunked(np.concatenate([fc1_w.T, fc2_w.T], 1).astype(f)),
        "bfc": np.concatenate([fc1_b, fc2_b])[None, :].astype(f),
        "eye": np.eye(128, dtype=f),
        "ones1": np.ones((1, 128), f),
    }
    oh = (types[:, :, None] == np.arange(NVT)[None, None, :]).astype(f)  # [B,NV,NVT]
    in_maps = []
    for c in range(NCORES):
        s = slice(c * P, (c + 1) * P)
        xt = oh[s].transpose(2, 1, 0).reshape(NVT, NV * P)           # [16, NV*P]
        xt1 = np.concatenate([xt, np.ones((1, NV * P), f)], 0)
        xp = params[s].transpose(2, 1, 0).reshape(FS, NV * P).astype(f)
        xp1 = np.concatenate([xp, np.ones((1, NV * P), f)], 0)
        m = dict(shared)
        m["xt1"] = np.ascontiguousarray(xt1)
        m["xp1"] = np.ascontiguousarray(xp1)
        m["adjt"] = np.ascontiguousarray(adj[s].reshape(P, NV * NV)).astype(f)
        in_maps.append(m)
    return in_maps


_NC_CACHE = {}


def _get_nc():
    key = str(MMDT)
    if key not in _NC_CACHE:
        _NC_CACHE[key] = build_bass()
    return _NC_CACHE[key]


F32_INPUTS = {"adjt", "eye"}


def kernel(**inputs):
    np_inputs = {k: np.asarray(v) for k, v in inputs.items()}
    in_maps = _host_prep(**np_inputs)
    npdt = mybir.dt.np(MMDT)
    if npdt != np.float32:
        in_maps = [{k: (v if k in F32_INPUTS else v.astype(npdt))
                    for k, v in m.items()} for m in in_maps]
    nc = _get_nc()
    res = run_bass_kernel_spmd(nc, in_maps, core_ids=list(range(NCORES)),
                               **_RUN_KWARGS)
    out = np.concatenate([res.results[c]["out"] for c in range(NCORES)], 0)
    _LAST_RESULT.clear()
    _LAST_RESULT.append(res)
    return out[:, :NZ], out[:, NZ:]


# test.py can set these to enable tracing / inspect results
_RUN_KWARGS = {}
_LAST_RESULT = []
